# revision 40
# baseline (speedup 1.0000x reference)
"""Trainium2 kernel for nn_IteratedLinearNet: y = x @ (W.T)^60.

Strategy (8 NeuronCores, single SPMD launch), v2 — single AllGather:
  - A := W.T. Each core j owns a 256-wide column slab S = A^p[:, Sj].
  - Crawl: S advances +1 per product (lhsT = W resident in SBUF):
    A^2, A^3, ..., A^8.
  - The A^8 slab is transposed on TensorE and AllGathered (the ONLY
    collective: 2 halves x 8MB) while +1 fill products A^9..A^12 keep
    TensorE busy.
  - (A^8)^T replaces W in the big SBUF buffer (tile-chased DMA), then
    +8 tail products: A^20, A^28, ..., A^60  (12 + 6*8 = 60).
  - Apply: ytj = (A^60 slab)^T-stationary x x^T streamed from HBM in
    bf16 1024-col chunks (2KB DMA lines), first chunks prefetched
    during the tail; the A^60 slab is cast to bf16 in the psum drain.
  - Chain matmuls in float32r (FP22-truncated reads, full PE rate);
    W pre-rounded to FP22-nearest on host; x rounded to bf16 (both are
    one-shot input roundings, no compounding through the power chain).

Self-contained: builds/compiles on first call and caches the module.
"""

import numpy as np

_GRID = 2048
_BATCH = 4096
_NCORES = 8
_SW = _GRID // _NCORES  # 256
_KT = _GRID // 128  # 16
_HALF = _GRID // 2

_N_FILL = 4  # +1 products that hide the AllGather (A^9..A^12)
_N_TAIL = 6  # +8 products (A^20..A^60)

_cache = {}


def _build():
    from contextlib import ExitStack

    import concourse.tile as tile
    from concourse import bacc, masks, mybir

    F32R = mybir.dt.float32r
    F32 = mybir.dt.float32
    BF16 = mybir.dt.bfloat16
    G, KT, SW, HALF, BATCH = _GRID, _KT, _SW, _HALF, _BATCH

    nc = bacc.Bacc(None, target_bir_lowering=False, num_devices=_NCORES)
    wt = nc.declare_dram_parameter("wt", [G, G], F32R, isOutput=False)
    aslab = nc.declare_dram_parameter("aslab", [G, SW], F32R, isOutput=False)
    xt = nc.declare_dram_parameter("xt", [G, BATCH], BF16, isOutput=False)
    ytj = nc.declare_dram_parameter("ytj", [SW, BATCH], F32R, isOutput=True)

    rg = [list(range(_NCORES))]

    with ExitStack() as ctx:
        tc = ctx.enter_context(tile.TileContext(nc))
        big = ctx.enter_context(tc.tile_pool(name="big", bufs=1))
        slabs = ctx.enter_context(tc.tile_pool(name="slabs", bufs=1))
        shpool = ctx.enter_context(tc.tile_pool(name="shpool", bufs=1))
        xpool = ctx.enter_context(tc.tile_pool(name="xpool", bufs=3))
        ypool = ctx.enter_context(tc.tile_pool(name="ypool", bufs=2))
        mmps = ctx.enter_context(tc.tile_pool(name="mmps", bufs=6, space="PSUM"))
        tps = ctx.enter_context(tc.tile_pool(name="tps", bufs=2, space="PSUM"))
        dram = ctx.enter_context(tc.tile_pool(name="dram", bufs=1, space="DRAM"))

        # big stationary operand: W, later (A^8)^T  (16MB)
        L = big.tile([128, KT, G], F32R)
        # slab ping-pong buffers (2MB each)
        sbuf = [
            slabs.tile([128, KT, SW], F32R, name=f"slab{i}", tag=f"slab{i}")
            for i in range(2)
        ]
        final_bf = slabs.tile([128, KT, SW], BF16, name="final_bf", tag="final_bf")
        ident32 = slabs.tile([128, 128], F32, name="ident32", tag="ident32")
        masks.make_identity(nc, ident32[:])
        ident = slabs.tile([128, 128], F32R, name="ident", tag="ident")
        nc.vector.tensor_copy(ident[:], ident32[:])

        # load the A^1 slab first (small) so P1's first matmuls can chase
        # the streaming W load
        for k in range(KT):
            nc.sync.dma_start(sbuf[0][:, k, :], aslab[128 * k : 128 * (k + 1), :])
        for k in range(KT):
            nc.sync.dma_start(L[:, k, :], wt[128 * k : 128 * (k + 1), :])

        def product(src, dst):
            """dst = M @ src where lhsT tiles come from L."""
            for m in range(KT):
                ps = mmps.tile([128, SW], F32, name="ps", tag="ps")
                for k in range(KT):
                    nc.tensor.matmul(
                        ps[:],
                        L[:, k, 128 * m : 128 * (m + 1)],
                        src[:, k, :],
                        start=(k == 0),
                        stop=(k == KT - 1),
                    )
                nc.vector.tensor_copy(dst[:, m, :], ps[:])

        # crawl: A^2 .. A^8  (7 products, lhsT = W)
        cur = 0
        for _ in range(7):
            product(sbuf[cur], sbuf[1 - cur])
            cur = 1 - cur
        a8 = sbuf[cur]  # A^8 slab

        # transpose A^8 slab per column half, stage to DRAM, AllGather
        ag_outs = []
        for h in range(2):
            t_sb = shpool.tile([128, 2, HALF], F32R, name=f"t_sb{h}", tag="t_sb")
            for k in range(8 * h, 8 * h + 8):
                for a in range(2):
                    psT = tps.tile([128, 128], F32R, name="psT", tag="psT")
                    nc.tensor.transpose(
                        psT[:], a8[:, k, 128 * a : 128 * (a + 1)], ident[:]
                    )
                    nc.vector.tensor_copy(
                        t_sb[:, a, 128 * (k - 8 * h) : 128 * (k - 8 * h + 1)],
                        psT[:],
                    )
            ag_in = dram.tile([SW, HALF], F32R, name=f"agin{h}", tag=f"agin{h}")
            for a in range(2):
                nc.sync.dma_start(
                    ag_in[128 * a : 128 * (a + 1), :],
                    t_sb[:, a, :],
                )
            ag_out = dram.tile(
                [G, HALF],
                F32R,
                name=f"agout{h}",
                tag=f"agout{h}",
                addr_space="Shared",
            )
            nc.gpsimd.collective_compute(
                "AllGather",
                mybir.AluOpType.bypass,
                replica_groups=rg,
                ins=[ag_in.opt()],
                outs=[ag_out.opt()],
            )
            ag_outs.append(ag_out)

        # prefetch the first apply x-chunks while the tail runs (the sync
        # queue is idle then; 3 slots = 3MB of the 16MB x^T stream)
        CB = 1024
        xpre = []
        for kq in range(3):
            xchunk = xpool.tile([128, 4, CB], BF16, name=f"xpre{kq}", tag="xchunk")
            for kk in range(4):
                k = 4 * kq + kk
                nc.sync.dma_start(
                    xchunk[:, kk, :], xt[128 * k : 128 * (k + 1), 0:CB]
                )
            xpre.append(xchunk)

        # fill products A^9..A^12 (+1, lhsT = W) while the AllGather runs
        for _ in range(_N_FILL):
            product(sbuf[cur], sbuf[1 - cur])
            cur = 1 - cur

        # load (A^8)^T into L, evicting W (tile-chased behind A^12's reads)
        for h in range(2):
            for k in range(KT):
                nc.sync.dma_start(
                    L[:, k, HALF * h : HALF * (h + 1)],
                    ag_outs[h][128 * k : 128 * (k + 1), :],
                )

        # tail products A^20 .. A^60 (+8, lhsT = (A^8)^T); the last one
        # writes the A^60 slab in bf16 for the bf16 apply
        for _ in range(_N_TAIL - 1):
            product(sbuf[cur], sbuf[1 - cur])
            cur = 1 - cur
        product(sbuf[cur], final_bf)
        final = final_bf  # A^60 slab (bf16)

        # apply: ytj[Sj, BATCH] = (A^60[:, Sj])^T @ x^T, x^T streamed in
        # bf16 1024-col chunks (2KB DMA lines, full HBM BW)
        for c in range(BATCH // CB):
            pss = [
                mmps.tile([128, 512], F32, name=f"psy{af}", tag="ps")
                for af in range(4)  # (a, f) = (af//2, af%2)
            ]
            for kq in range(4):
                if c == 0 and kq < 3:
                    xchunk = xpre[kq]
                else:
                    xchunk = xpool.tile(
                        [128, 4, CB], BF16, name="xchunk", tag="xchunk"
                    )
                    for kk in range(4):
                        k = 4 * kq + kk
                        nc.sync.dma_start(
                            xchunk[:, kk, :],
                            xt[128 * k : 128 * (k + 1), CB * c : CB * (c + 1)],
                        )
                for kk in range(4):
                    k = 4 * kq + kk
                    for a in range(2):
                        for f in range(2):
                            nc.tensor.matmul(
                                pss[2 * a + f][:],
                                final[:, k, 128 * a : 128 * (a + 1)],
                                xchunk[:, kk, 512 * f : 512 * (f + 1)],
                                start=(k == 0),
                                stop=(k == KT - 1),
                            )
            for a in range(2):
                for f in range(2):
                    ystage = ypool.tile([128, 512], F32R, name="ystage", tag="ystage")
                    nc.vector.tensor_copy(ystage[:], pss[2 * a + f][:])
                    nc.scalar.dma_start(
                        ytj[
                            128 * a : 128 * (a + 1),
                            CB * c + 512 * f : CB * c + 512 * (f + 1),
                        ],
                        ystage[:],
                    )
    nc.compile()
    return nc


def _round22(a):
    bits = np.ascontiguousarray(a).view(np.uint32)
    return ((bits + 0x200) & np.uint32(0xFFFFFC00)).view(np.float32)


def kernel(x, W):
    from concourse.bass_utils import run_bass_kernel_spmd

    if "nc" not in _cache:
        _cache["nc"] = _build()
    nc = _cache["nc"]

    import ml_dtypes

    Wr = _round22(np.asarray(W, dtype=np.float32))
    wt_np = np.ascontiguousarray(Wr)
    xt_np = np.ascontiguousarray(
        np.asarray(x, dtype=np.float32).T.astype(ml_dtypes.bfloat16)
    )
    in_maps = [
        {
            "wt": wt_np,
            "aslab": np.ascontiguousarray(Wr[_SW * j : _SW * (j + 1), :].T),
            "xt": xt_np,
        }
        for j in range(_NCORES)
    ]
    res = run_bass_kernel_spmd(nc, in_maps, core_ids=list(range(_NCORES)))
    _cache["last_exec_time_ns"] = res.exec_time_ns
    _cache["last_results"] = res
    y = np.concatenate(
        [res.results[j]["ytj"].T for j in range(_NCORES)], axis=1
    ).astype(np.float32)
    return y


# revision 42
# speedup vs baseline: 1.0101x; 1.0101x over previous
"""Trainium2 kernel for nn_IteratedLinearNet: y = x @ (W.T)^60.

Strategy (8 NeuronCores, single SPMD launch), v2 — single AllGather:
  - A := W.T. Each core j owns a 256-wide column slab S = A^p[:, Sj].
  - Crawl: S advances +1 per product (lhsT = W resident in SBUF):
    A^2, A^3, ..., A^8.
  - The A^8 slab is transposed on TensorE and AllGathered (the ONLY
    collective: 2 halves x 8MB) while +1 fill products A^9..A^12 keep
    TensorE busy.
  - (A^8)^T replaces W in the big SBUF buffer (tile-chased DMA), then
    +8 tail products: A^20, A^28, ..., A^60  (12 + 6*8 = 60).
  - Apply: ytj = (A^60 slab)^T-stationary x x^T streamed from HBM in
    bf16 1024-col chunks (2KB DMA lines), first chunks prefetched
    during the tail; the A^60 slab is cast to bf16 in the psum drain.
  - Chain matmuls in float32r (FP22-truncated reads, full PE rate);
    W pre-rounded to FP22-nearest on host; x rounded to bf16 (both are
    one-shot input roundings, no compounding through the power chain).

Self-contained: builds/compiles on first call and caches the module.
"""

import numpy as np

_GRID = 2048
_BATCH = 4096
_NCORES = 8
_SW = _GRID // _NCORES  # 256
_KT = _GRID // 128  # 16
_HALF = _GRID // 2

_N_FILL = 4  # +1 products that hide the AllGather (A^9..A^12)
_N_TAIL = 6  # +8 products (A^20..A^60)

_cache = {}


def _build():
    from contextlib import ExitStack

    import concourse.tile as tile
    from concourse import bacc, masks, mybir

    F32R = mybir.dt.float32r
    F32 = mybir.dt.float32
    BF16 = mybir.dt.bfloat16
    G, KT, SW, HALF, BATCH = _GRID, _KT, _SW, _HALF, _BATCH

    nc = bacc.Bacc(None, target_bir_lowering=False, num_devices=_NCORES)
    wt = nc.declare_dram_parameter("wt", [G, G], F32R, isOutput=False)
    aslab = nc.declare_dram_parameter("aslab", [G, SW], F32R, isOutput=False)
    xt = nc.declare_dram_parameter("xt", [G, BATCH], BF16, isOutput=False)
    ytj = nc.declare_dram_parameter("ytj", [SW, BATCH], F32R, isOutput=True)

    rg = [list(range(_NCORES))]

    with ExitStack() as ctx:
        tc = ctx.enter_context(tile.TileContext(nc))
        big = ctx.enter_context(tc.tile_pool(name="big", bufs=1))
        slabs = ctx.enter_context(tc.tile_pool(name="slabs", bufs=1))
        shpool = ctx.enter_context(tc.tile_pool(name="shpool", bufs=1))
        xpool = ctx.enter_context(tc.tile_pool(name="xpool", bufs=3))
        ypool = ctx.enter_context(tc.tile_pool(name="ypool", bufs=3))
        mmps = ctx.enter_context(tc.tile_pool(name="mmps", bufs=6, space="PSUM"))
        tps = ctx.enter_context(tc.tile_pool(name="tps", bufs=2, space="PSUM"))
        dram = ctx.enter_context(tc.tile_pool(name="dram", bufs=1, space="DRAM"))

        # big stationary operand: W, later (A^8)^T  (16MB)
        L = big.tile([128, KT, G], F32R)
        # slab ping-pong buffers (2MB each)
        sbuf = [
            slabs.tile([128, KT, SW], F32R, name=f"slab{i}", tag=f"slab{i}")
            for i in range(2)
        ]
        final_bf = slabs.tile([128, KT, SW], BF16, name="final_bf", tag="final_bf")
        ident32 = slabs.tile([128, 128], F32, name="ident32", tag="ident32")
        masks.make_identity(nc, ident32[:])
        ident = slabs.tile([128, 128], F32R, name="ident", tag="ident")
        nc.vector.tensor_copy(ident[:], ident32[:])

        # load the A^1 slab first (small) so P1's first matmuls can chase
        # the streaming W load
        for k in range(KT):
            nc.sync.dma_start(sbuf[0][:, k, :], aslab[128 * k : 128 * (k + 1), :])
        for k in range(KT):
            nc.sync.dma_start(L[:, k, :], wt[128 * k : 128 * (k + 1), :])

        def product(src, dst):
            """dst = M @ src where lhsT tiles come from L."""
            for m in range(KT):
                ps = mmps.tile([128, SW], F32, name="ps", tag="ps")
                for k in range(KT):
                    nc.tensor.matmul(
                        ps[:],
                        L[:, k, 128 * m : 128 * (m + 1)],
                        src[:, k, :],
                        start=(k == 0),
                        stop=(k == KT - 1),
                    )
                nc.vector.tensor_copy(dst[:, m, :], ps[:])

        # crawl: A^2 .. A^8  (7 products, lhsT = W)
        cur = 0
        for _ in range(7):
            product(sbuf[cur], sbuf[1 - cur])
            cur = 1 - cur
        a8 = sbuf[cur]  # A^8 slab

        # transpose A^8 slab per column half, stage to DRAM, AllGather
        ag_outs = []
        for h in range(2):
            t_sb = shpool.tile([128, 2, HALF], F32R, name=f"t_sb{h}", tag="t_sb")
            for k in range(8 * h, 8 * h + 8):
                for a in range(2):
                    psT = tps.tile([128, 128], F32R, name="psT", tag="psT")
                    nc.tensor.transpose(
                        psT[:], a8[:, k, 128 * a : 128 * (a + 1)], ident[:]
                    )
                    nc.vector.tensor_copy(
                        t_sb[:, a, 128 * (k - 8 * h) : 128 * (k - 8 * h + 1)],
                        psT[:],
                    )
            ag_in = dram.tile([SW, HALF], F32R, name=f"agin{h}", tag=f"agin{h}")
            for a in range(2):
                nc.sync.dma_start(
                    ag_in[128 * a : 128 * (a + 1), :],
                    t_sb[:, a, :],
                )
            ag_out = dram.tile(
                [G, HALF],
                F32R,
                name=f"agout{h}",
                tag=f"agout{h}",
                addr_space="Shared",
            )
            nc.gpsimd.collective_compute(
                "AllGather",
                mybir.AluOpType.bypass,
                replica_groups=rg,
                ins=[ag_in.opt()],
                outs=[ag_out.opt()],
            )
            ag_outs.append(ag_out)

        # prefetch the first apply x-chunks while the tail runs (the sync
        # queue is idle then; 3 slots = 3MB of the 16MB x^T stream)
        CB = 1024
        xpre = []
        for kq in range(3):
            xchunk = xpool.tile([128, 4, CB], BF16, name=f"xpre{kq}", tag="xchunk")
            for kk in range(4):
                k = 4 * kq + kk
                nc.sync.dma_start(
                    xchunk[:, kk, :], xt[128 * k : 128 * (k + 1), 0:CB]
                )
            xpre.append(xchunk)

        # fill products A^9..A^12 (+1, lhsT = W) while the AllGather runs
        for _ in range(_N_FILL):
            product(sbuf[cur], sbuf[1 - cur])
            cur = 1 - cur

        # load (A^8)^T into L, evicting W (tile-chased behind A^12's reads)
        for h in range(2):
            for k in range(KT):
                nc.sync.dma_start(
                    L[:, k, HALF * h : HALF * (h + 1)],
                    ag_outs[h][128 * k : 128 * (k + 1), :],
                )

        # tail products A^20 .. A^60 (+8, lhsT = (A^8)^T); the last one
        # writes the A^60 slab in bf16 for the bf16 apply
        for _ in range(_N_TAIL - 1):
            product(sbuf[cur], sbuf[1 - cur])
            cur = 1 - cur
        product(sbuf[cur], final_bf)
        final = final_bf  # A^60 slab (bf16)

        # apply: ytj[Sj, BATCH] = (A^60[:, Sj])^T @ x^T, x^T streamed in
        # bf16 1024-col chunks (2KB DMA lines, full HBM BW)
        for c in range(BATCH // CB):
            pss = [
                mmps.tile([128, 512], F32, name=f"psy{af}", tag="ps")
                for af in range(4)  # (a, f) = (af//2, af%2)
            ]
            for kq in range(4):
                if c == 0 and kq < 3:
                    xchunk = xpre[kq]
                else:
                    xchunk = xpool.tile(
                        [128, 4, CB], BF16, name="xchunk", tag="xchunk"
                    )
                    for kk in range(4):
                        k = 4 * kq + kk
                        nc.sync.dma_start(
                            xchunk[:, kk, :],
                            xt[128 * k : 128 * (k + 1), CB * c : CB * (c + 1)],
                        )
                for kk in range(4):
                    k = 4 * kq + kk
                    for a in range(2):
                        for f in range(2):
                            nc.tensor.matmul(
                                pss[2 * a + f][:],
                                final[:, k, 128 * a : 128 * (a + 1)],
                                xchunk[:, kk, 512 * f : 512 * (f + 1)],
                                start=(k == 0),
                                stop=(k == KT - 1),
                            )
            for a in range(2):
                for f in range(2):
                    ystage = ypool.tile([128, 512], F32R, name="ystage", tag="ystage")
                    nc.vector.tensor_copy(ystage[:], pss[2 * a + f][:])
                    nc.scalar.dma_start(
                        ytj[
                            128 * a : 128 * (a + 1),
                            CB * c + 512 * f : CB * c + 512 * (f + 1),
                        ],
                        ystage[:],
                    )
    nc.compile()
    return nc


def _round22(a):
    bits = np.ascontiguousarray(a).view(np.uint32)
    return ((bits + 0x200) & np.uint32(0xFFFFFC00)).view(np.float32)


def kernel(x, W):
    from concourse.bass_utils import run_bass_kernel_spmd

    if "nc" not in _cache:
        _cache["nc"] = _build()
    nc = _cache["nc"]

    import ml_dtypes

    Wr = _round22(np.asarray(W, dtype=np.float32))
    wt_np = np.ascontiguousarray(Wr)
    xt_np = np.ascontiguousarray(
        np.asarray(x, dtype=np.float32).T.astype(ml_dtypes.bfloat16)
    )
    in_maps = [
        {
            "wt": wt_np,
            "aslab": np.ascontiguousarray(Wr[_SW * j : _SW * (j + 1), :].T),
            "xt": xt_np,
        }
        for j in range(_NCORES)
    ]
    res = run_bass_kernel_spmd(nc, in_maps, core_ids=list(range(_NCORES)))
    _cache["last_exec_time_ns"] = res.exec_time_ns
    _cache["last_results"] = res
    y = np.concatenate(
        [res.results[j]["ytj"].T for j in range(_NCORES)], axis=1
    ).astype(np.float32)
    return y


# revision 43
# speedup vs baseline: 1.0216x; 1.0113x over previous
"""Trainium2 kernel for nn_IteratedLinearNet: y = x @ (W.T)^60.

Strategy (8 NeuronCores, single SPMD launch), v2 — single AllGather:
  - A := W.T. Each core j owns a 256-wide column slab S = A^p[:, Sj].
  - Crawl: S advances +1 per product (lhsT = W resident in SBUF):
    A^2, A^3, ..., A^8.
  - The A^8 slab is transposed on TensorE and AllGathered (the ONLY
    collective: 2 halves x 8MB) while +1 fill products A^9..A^12 keep
    TensorE busy.
  - (A^8)^T replaces W in the big SBUF buffer (tile-chased DMA), then
    +8 tail products: A^20, A^28, ..., A^60  (12 + 6*8 = 60).
  - Apply: ytj = (A^60 slab)^T-stationary x x^T streamed from HBM in
    bf16 1024-col chunks (2KB DMA lines), first chunks prefetched
    during the tail; the A^60 slab is cast to bf16 in the psum drain.
  - Chain matmuls in float32r (FP22-truncated reads, full PE rate);
    W pre-rounded to FP22-nearest on host; x rounded to bf16 (both are
    one-shot input roundings, no compounding through the power chain).

Self-contained: builds/compiles on first call and caches the module.
"""

import numpy as np

_GRID = 2048
_BATCH = 4096
_NCORES = 8
_SW = _GRID // _NCORES  # 256
_KT = _GRID // 128  # 16
_HALF = _GRID // 2

_N_FILL = 4  # +1 products that hide the AllGather (A^9..A^12)
_N_TAIL = 6  # +8 products (A^20..A^60)

_cache = {}


def _build():
    from contextlib import ExitStack

    import concourse.tile as tile
    from concourse import bacc, masks, mybir

    F32R = mybir.dt.float32r
    F32 = mybir.dt.float32
    BF16 = mybir.dt.bfloat16
    G, KT, SW, HALF, BATCH = _GRID, _KT, _SW, _HALF, _BATCH

    nc = bacc.Bacc(None, target_bir_lowering=False, num_devices=_NCORES)
    wt = nc.declare_dram_parameter("wt", [G, G], F32R, isOutput=False)
    aslab = nc.declare_dram_parameter("aslab", [G, SW], F32R, isOutput=False)
    xt = nc.declare_dram_parameter("xt", [G, BATCH], BF16, isOutput=False)
    ytj = nc.declare_dram_parameter("ytj", [SW, BATCH], F32R, isOutput=True)

    rg = [list(range(_NCORES))]

    with ExitStack() as ctx:
        tc = ctx.enter_context(tile.TileContext(nc))
        big = ctx.enter_context(tc.tile_pool(name="big", bufs=1))
        slabs = ctx.enter_context(tc.tile_pool(name="slabs", bufs=1))
        shpool = ctx.enter_context(tc.tile_pool(name="shpool", bufs=1))
        xpool = ctx.enter_context(tc.tile_pool(name="xpool", bufs=3))
        ypool = ctx.enter_context(tc.tile_pool(name="ypool", bufs=3))
        mmps = ctx.enter_context(tc.tile_pool(name="mmps", bufs=6, space="PSUM"))
        tps = ctx.enter_context(tc.tile_pool(name="tps", bufs=2, space="PSUM"))
        dram = ctx.enter_context(tc.tile_pool(name="dram", bufs=1, space="DRAM"))

        # big stationary operand: W, later (A^8)^T  (16MB)
        L = big.tile([128, KT, G], F32R)
        # slab ping-pong buffers (2MB each)
        sbuf = [
            slabs.tile([128, KT, SW], F32R, name=f"slab{i}", tag=f"slab{i}")
            for i in range(2)
        ]
        final_bf = slabs.tile([128, KT, SW], BF16, name="final_bf", tag="final_bf")
        ident32 = slabs.tile([128, 128], F32, name="ident32", tag="ident32")
        masks.make_identity(nc, ident32[:])
        ident = slabs.tile([128, 128], F32R, name="ident", tag="ident")
        nc.vector.tensor_copy(ident[:], ident32[:])

        # load the A^1 slab first (small) so P1's first matmuls can chase
        # the streaming W load
        for k in range(KT):
            nc.scalar.dma_start(sbuf[0][:, k, :], aslab[128 * k : 128 * (k + 1), :])
        for k in range(KT):
            nc.sync.dma_start(L[:, k, :], wt[128 * k : 128 * (k + 1), :])

        def product(src, dst):
            """dst = M @ src where lhsT tiles come from L."""
            for m in range(KT):
                ps = mmps.tile([128, SW], F32, name="ps", tag="ps")
                for k in range(KT):
                    nc.tensor.matmul(
                        ps[:],
                        L[:, k, 128 * m : 128 * (m + 1)],
                        src[:, k, :],
                        start=(k == 0),
                        stop=(k == KT - 1),
                    )
                nc.vector.tensor_copy(dst[:, m, :], ps[:])

        # crawl: A^2 .. A^8  (7 products, lhsT = W)
        cur = 0
        for _ in range(7):
            product(sbuf[cur], sbuf[1 - cur])
            cur = 1 - cur
        a8 = sbuf[cur]  # A^8 slab

        # transpose A^8 slab per column half, stage to DRAM, AllGather
        ag_outs = []
        for h in range(2):
            t_sb = shpool.tile([128, 2, HALF], F32R, name=f"t_sb{h}", tag="t_sb")
            for k in range(8 * h, 8 * h + 8):
                for a in range(2):
                    psT = tps.tile([128, 128], F32R, name="psT", tag="psT")
                    nc.tensor.transpose(
                        psT[:], a8[:, k, 128 * a : 128 * (a + 1)], ident[:]
                    )
                    nc.vector.tensor_copy(
                        t_sb[:, a, 128 * (k - 8 * h) : 128 * (k - 8 * h + 1)],
                        psT[:],
                    )
            ag_in = dram.tile([SW, HALF], F32R, name=f"agin{h}", tag=f"agin{h}")
            for a in range(2):
                nc.sync.dma_start(
                    ag_in[128 * a : 128 * (a + 1), :],
                    t_sb[:, a, :],
                )
            ag_out = dram.tile(
                [G, HALF],
                F32R,
                name=f"agout{h}",
                tag=f"agout{h}",
                addr_space="Shared",
            )
            nc.gpsimd.collective_compute(
                "AllGather",
                mybir.AluOpType.bypass,
                replica_groups=rg,
                ins=[ag_in.opt()],
                outs=[ag_out.opt()],
            )
            ag_outs.append(ag_out)

        # prefetch the first apply x-chunks while the tail runs (the sync
        # queue is idle then; 3 slots = 3MB of the 16MB x^T stream)
        CB = 1024
        xpre = []
        for kq in range(3):
            xchunk = xpool.tile([128, 4, CB], BF16, name=f"xpre{kq}", tag="xchunk")
            for kk in range(4):
                k = 4 * kq + kk
                nc.sync.dma_start(
                    xchunk[:, kk, :], xt[128 * k : 128 * (k + 1), 0:CB]
                )
            xpre.append(xchunk)

        # fill products A^9..A^12 (+1, lhsT = W) while the AllGather runs
        for _ in range(_N_FILL):
            product(sbuf[cur], sbuf[1 - cur])
            cur = 1 - cur

        # load (A^8)^T into L, evicting W (tile-chased behind A^12's reads)
        for h in range(2):
            for k in range(KT):
                nc.sync.dma_start(
                    L[:, k, HALF * h : HALF * (h + 1)],
                    ag_outs[h][128 * k : 128 * (k + 1), :],
                )

        # tail products A^20 .. A^60 (+8, lhsT = (A^8)^T); the last one
        # writes the A^60 slab in bf16 for the bf16 apply
        for _ in range(_N_TAIL - 1):
            product(sbuf[cur], sbuf[1 - cur])
            cur = 1 - cur
        product(sbuf[cur], final_bf)
        final = final_bf  # A^60 slab (bf16)

        # apply: ytj[Sj, BATCH] = (A^60[:, Sj])^T @ x^T, x^T streamed in
        # bf16 1024-col chunks (2KB DMA lines, full HBM BW)
        for c in range(BATCH // CB):
            pss = [
                mmps.tile([128, 512], F32, name=f"psy{af}", tag="ps")
                for af in range(4)  # (a, f) = (af//2, af%2)
            ]
            for kq in range(4):
                if c == 0 and kq < 3:
                    xchunk = xpre[kq]
                else:
                    xchunk = xpool.tile(
                        [128, 4, CB], BF16, name="xchunk", tag="xchunk"
                    )
                    for kk in range(4):
                        k = 4 * kq + kk
                        nc.sync.dma_start(
                            xchunk[:, kk, :],
                            xt[128 * k : 128 * (k + 1), CB * c : CB * (c + 1)],
                        )
                for kk in range(4):
                    k = 4 * kq + kk
                    for a in range(2):
                        for f in range(2):
                            nc.tensor.matmul(
                                pss[2 * a + f][:],
                                final[:, k, 128 * a : 128 * (a + 1)],
                                xchunk[:, kk, 512 * f : 512 * (f + 1)],
                                start=(k == 0),
                                stop=(k == KT - 1),
                            )
            for a in range(2):
                for f in range(2):
                    ystage = ypool.tile([128, 512], F32R, name="ystage", tag="ystage")
                    nc.vector.tensor_copy(ystage[:], pss[2 * a + f][:])
                    nc.scalar.dma_start(
                        ytj[
                            128 * a : 128 * (a + 1),
                            CB * c + 512 * f : CB * c + 512 * (f + 1),
                        ],
                        ystage[:],
                    )
    nc.compile()
    return nc


def _round22(a):
    bits = np.ascontiguousarray(a).view(np.uint32)
    return ((bits + 0x200) & np.uint32(0xFFFFFC00)).view(np.float32)


def kernel(x, W):
    from concourse.bass_utils import run_bass_kernel_spmd

    if "nc" not in _cache:
        _cache["nc"] = _build()
    nc = _cache["nc"]

    import ml_dtypes

    Wr = _round22(np.asarray(W, dtype=np.float32))
    wt_np = np.ascontiguousarray(Wr)
    xt_np = np.ascontiguousarray(
        np.asarray(x, dtype=np.float32).T.astype(ml_dtypes.bfloat16)
    )
    in_maps = [
        {
            "wt": wt_np,
            "aslab": np.ascontiguousarray(Wr[_SW * j : _SW * (j + 1), :].T),
            "xt": xt_np,
        }
        for j in range(_NCORES)
    ]
    res = run_bass_kernel_spmd(nc, in_maps, core_ids=list(range(_NCORES)))
    _cache["last_exec_time_ns"] = res.exec_time_ns
    _cache["last_results"] = res
    y = np.concatenate(
        [res.results[j]["ytj"].T for j in range(_NCORES)], axis=1
    ).astype(np.float32)
    return y
